# revision 1
# baseline (speedup 1.0000x reference)
"""Trainium2 Bass kernel for StyleGAN2-style upsampled Conv1d.

Reference computation (for x:(16,256,4096), weight:(256,256,3), bias:(256,)):
  y = conv_transpose1d(x, weight, stride=2)      # correlation on 2x-dilated x
  z = upfirdn1d(y, [1,3,3,1]/8 * 2)              # depthwise FIR
  out = z + bias                                  # (16, 256, 8192)

The transposed conv + FIR collapse into TWO 3-tap correlations over the
original x grid (even/odd output phases):
  out[:, :, 2j]   = A @x[j-1] + B @x[j]  + C @x[j+1]
  out[:, :, 2j+1] = A'@x[j-1] + B'@x[j]  + C'@x[j+1]
with (w0,w1,w2) = weight taps:
  A  = .75 w0 + .25 w1   B  = .25 w0 + .75 w1 + .75 w2   C  = .25 w2
  A' = .25 w0            B' = .75 w0 + .75 w1 + .25 w2   C' = .25 w1 + .75 w2

On-chip: each 3-tap correlation is 6 accumulating fp32r matmuls (3 taps x
2 K-tiles of 128) into one PSUM bank per 512-position chunk; even/odd
banks are paired so one vector/scalar op per pair drains PSUM, adds bias,
and interleaves the two phases into the final layout.  Sharding:
data-parallel over batch (2 per core x 8 cores).
"""

import numpy as np

import concourse.bass as bass
import concourse.mybir as mybir
import concourse.tile as tile
from concourse import bacc
from concourse.bass_utils import run_bass_kernel_spmd

N, IN_CH, OUT_CH, KERNEL, D = 16, 256, 256, 3, 4096
NCORES = 8
BPC = N // NCORES          # batches per core
DOUT = 2 * D
F32 = mybir.dt.float32
F32R = mybir.dt.float32r

NCHUNK = 512               # matmul moving free dim (= one PSUM bank of fp32)
NCHUNKS = D // NCHUNK      # 8
GROUP = 4                  # psum pairs accumulated concurrently (4 pairs = 8 banks)

_CACHED = {}


def _wblk(phase, tap, k, m):
    return ((phase * 3 + tap) * 2 + k) * 2 + m


def _build_nc(mm_dtype=F32R):
    nc = bacc.Bacc("TRN2", target_bir_lowering=False, debug=False)

    # x arrives host-padded with zero columns at 0 and D+1 (3-tap halo).
    x_t = nc.dram_tensor("x", [BPC, IN_CH, D + 2], F32, kind="ExternalInput")
    # w layout: 24 blocks of (128 K, 128 M); see _wblk
    w_t = nc.dram_tensor("w", [128, 24 * 128], F32, kind="ExternalInput")
    b_t = nc.dram_tensor("b", [128, 2], F32, kind="ExternalInput")
    o_t = nc.dram_tensor("out", [BPC, OUT_CH, DOUT], F32, kind="ExternalOutput")

    with tile.TileContext(nc) as tc:
        with (
            tc.tile_pool(name="wpool", bufs=1) as wpool,
            tc.tile_pool(name="xpool", bufs=2 * BPC) as xpool,
            tc.tile_pool(name="zpool", bufs=6) as zpool,
            tc.tile_pool(name="ppool", bufs=GROUP, space="PSUM") as ppool,
        ):
            w_sb = wpool.tile([128, 24 * 128], mm_dtype)
            nc.gpsimd.dma_start(out=w_sb[:], in_=w_t[:])
            b_sb = wpool.tile([128, 2], F32)
            nc.sync.dma_start(out=b_sb[:], in_=b_t[:])

            # x tiles (128, D+2), cast fp32 -> fp32r by the SWDGE DMAs.
            # Two column-block DMAs per tile; the SWDGE queue is FIFO, so
            # load the first blocks of BOTH K-tiles before any second block
            # (the first matmul group reads both).
            half = GROUP * NCHUNK + 3  # covers chunk group 0 reads
            x_sb = {}
            for bb in range(BPC):
                for k in range(2):
                    x_sb[bb, k] = xpool.tile(
                        [128, D + 2], mm_dtype, tag="x", name=f"x_{bb}_{k}"
                    )
            for bb in range(BPC):
                for blk, (lo, hi) in enumerate([(0, half), (half, D + 2)]):
                    for k in range(2):
                        nc.gpsimd.dma_start(
                            out=x_sb[bb, k][:, lo:hi],
                            in_=x_t[bb, k * 128:(k + 1) * 128, lo:hi],
                        )

            # Pre-warm the PE while inputs load: dummy bf16 matmuls on a
            # memset tile (no DMA dependency -- they start right after the
            # preamble) flip the HAM clock gate to 8/8 and keep it warm
            # until the real work arrives.  The PSUM garbage lands in a
            # pool slot that a real accumulation group's start=True clears.
            warm_bf = wpool.tile([128, 128 + NCHUNK], mybir.dt.bfloat16)
            nc.vector.memset(warm_bf[:], 1.0)
            warm_ps = ppool.tile([128, 2 * NCHUNK], F32, tag="pair", name="warm_ps")
            for _ in range(36):
                nc.tensor.matmul(
                    warm_ps[:, 0:NCHUNK],
                    lhsT=warm_bf[:, 0:128],
                    rhs=warm_bf[:, 128:128 + NCHUNK],
                    start=True,
                    stop=True,
                )

            for bb in range(BPC):
                for m in range(2):
                    bias_ap = b_sb[:, m:m + 1]
                    for g in range(NCHUNKS // GROUP):
                        pairs = [
                            ppool.tile([128, 2 * NCHUNK], F32, tag="pair",
                                       name=f"pair_{bb}_{m}_{g}_{i}")
                            for i in range(GROUP)
                        ]
                        # weight-stationary inner order: each of the 12
                        # (phase,tap,ktile) weights streams GROUP chunks.
                        for phase in range(2):
                            for tap in range(3):
                                for k in range(2):
                                    w_ap = w_sb[:, _wblk(phase, tap, k, m) * 128:][:, :128]
                                    for ci in range(GROUP):
                                        c = g * GROUP + ci
                                        rhs = x_sb[bb, k][:, NCHUNK * c + tap:NCHUNK * c + tap + NCHUNK]
                                        nc.tensor.matmul(
                                            pairs[ci][:, phase * NCHUNK:(phase + 1) * NCHUNK],
                                            lhsT=w_ap,
                                            rhs=rhs,
                                            start=(tap == 0 and k == 0),
                                            stop=(tap == 2 and k == 1),
                                        )
                        for ci in range(GROUP):
                            c = g * GROUP + ci
                            zt = zpool.tile([128, 2 * NCHUNK], F32, tag="z",
                                            name=f"z_{bb}_{m}_{c}")
                            # psum pair is [even(512) | odd(512)]; writing in
                            # (phase, j) order at stride 2 interleaves the two
                            # phases while adding bias -- one op per pair,
                            # pairs alternating between vector and scalar.
                            vout = zt[:].rearrange("p (j two) -> p two j", two=2)
                            vin = pairs[ci][:].rearrange("p (two j) -> p two j", two=2)
                            if ci % 2 == 0:
                                nc.vector.tensor_scalar(
                                    out=vout, in0=vin,
                                    scalar1=bias_ap, scalar2=None,
                                    op0=mybir.AluOpType.add,
                                )
                            else:
                                nc.scalar.activation(
                                    out=vout, in_=vin,
                                    func=mybir.ActivationFunctionType.Identity,
                                    bias=bias_ap,
                                )
                            # Final quadrant's outputs ride the by-then idle
                            # scalar HWDGE queue so the kernel tail is not
                            # serialized behind the sync queue's backlog.
                            oeng = nc.scalar if (bb == 1 and m == 1) else nc.sync
                            oeng.dma_start(
                                out=o_t[bb, m * 128:(m + 1) * 128,
                                        c * 2 * NCHUNK:(c + 1) * 2 * NCHUNK],
                                in_=zt[:],
                            )
    nc.compile()
    return nc


def _host_weights(weight, bias):
    w = np.asarray(weight, dtype=np.float32)
    w0, w1, w2 = w[:, :, 0], w[:, :, 1], w[:, :, 2]
    taps = [
        [0.75 * w0 + 0.25 * w1, 0.25 * w0 + 0.75 * w1 + 0.75 * w2, 0.25 * w2],
        [0.25 * w0, 0.75 * w0 + 0.75 * w1 + 0.25 * w2, 0.25 * w1 + 0.75 * w2],
    ]
    w_host = np.zeros((128, 24 * 128), dtype=np.float32)
    for phase in range(2):
        for tap in range(3):
            for k in range(2):
                for m in range(2):
                    blk = _wblk(phase, tap, k, m)
                    # lhsT block[i, o] = W[phase][tap][m*128+o, k*128+i]
                    wt = taps[phase][tap][m * 128:(m + 1) * 128, k * 128:(k + 1) * 128]
                    w_host[:, blk * 128:(blk + 1) * 128] = wt.T
    b_host = np.asarray(bias, dtype=np.float32).reshape(2, 128).T.copy()
    return w_host, b_host


def _host_x(x):
    x = np.asarray(x, dtype=np.float32)
    return np.ascontiguousarray(np.pad(x, ((0, 0), (0, 0), (1, 1))))


def kernel(x, weight, bias):
    x = _host_x(x)
    w_host, b_host = _host_weights(weight, bias)

    if "nc" not in _CACHED:
        _CACHED["nc"] = _build_nc()
    nc = _CACHED["nc"]

    in_maps = []
    for core in range(NCORES):
        shard = np.ascontiguousarray(x[core * BPC:(core + 1) * BPC])
        in_maps.append({"x": shard, "w": w_host, "b": b_host})

    res = run_bass_kernel_spmd(nc, in_maps, core_ids=list(range(NCORES)))
    out = np.concatenate([np.asarray(r["out"]) for r in res.results], axis=0)
    return out

